# revision 34
# baseline (speedup 1.0000x reference)
"""Multi-head causal attention (B=2, N=2048, D=1024, H=16) on 8 NeuronCores.

Sharding: tensor-parallel over heads — each core computes 2 heads end-to-end
(QKV projections for its 128 head-dims, attention, and the 128 output
columns of the final projection that its rank id owns).

Host<->device traffic is minimized with on-device collectives:
  - Each core uploads only its OWN 128-dim slice of x.T for q/k/v
    ([384, 4096] bf16, ~3 MB) plus its weight slices; two pipelined
    AllGathers (q+k fused, then v) reconstruct the full x.T in device
    DRAM on every core (x is shipped over PCIe exactly once, not 8
    times, and the q/k projections overlap the v gather).
  - After attention, the per-core ctx.T [128, 2048] (per batch) is
    AllGathered (1 MB/rank on-wire) so every core holds the full
    ctx.T [1024, tokens]; each core then computes output columns
    [c*128:(c+1)*128] exactly (f32 PSUM accumulation over all 1024 ctx
    dims) and downloads just that [128, 4096] bf16 slice. The host
    transposes/concats and adds bo. No partial-sum download, no host
    reduction.

Per-core device program (single NEFF, Tile framework, bf16 matmuls),
batch 0's ctx AllGather and output projection overlap batch 1's
attention:
  1. qT/kT/vT projections: stationary = W.T chunk [128dk,128pd], moving =
     x.T chunk [128dk, 2048seq] from the gathered x.T, accumulated over 8
     D-chunks block-by-block in a PSUM accumulator.
  2. vT -> v via PE transpose into v_aug[keys, vA|vB].
  3. Attention per 512-q block: for each 128-key chunk j:
       S.T = row-packed matmuls (head A on contraction partitions 0:64,
       head B on 64:128 — concurrent in the PE array),
       P.T = exp(scale*S.T) on ScalarE (scores are O(5), no max needed),
       causal diagonal tiles get a multiplicative triangular bf16 mask,
       O.T += v65.T@P.T per head where v65 carries a ones column, so the
       softmax denominator accumulates in PSUM row 64 of the same matmul.
     ctxT = O.T[0:64] * reciprocal(l replicated via a 1-row ones matmul);
     the replication tail is deferred into the next q-block's chunk loop
     so the in-order PE queue never waits on the l-row copy.
  4. ctx AllGather + column-sliced output projection (stationary = Wo.T
     chunk, moving = gathered ctx.T chunk) -> outT [128, 4096] -> DRAM.

The mask structure is detected on the host: causal and all-ones get fast
schedules; arbitrary masks fall back to multiplicative bf16 mask blocks.
"""

from contextlib import ExitStack

import numpy as np
import ml_dtypes

B, N, D, H = 2, 2048, 1024, 16
DK = D // H          # 64
NCORES = 8
HPC = H // NCORES    # 2 heads per core
PD = HPC * DK        # 128 dims per core
BN = B * N           # 4096
NKC = N // 128       # 16 key chunks per sequence
NQB = N // 512       # 4 q-blocks of 512 per sequence
SCALE = DK ** -0.5

BF16 = ml_dtypes.bfloat16


def _mask_schedule(mask):
    """Classify the [N,N] mask into a per-(qblock, keychunk) schedule.

    Returns (mode, sched, mask_pack). sched[qb] is a list of entries
    (j, d0, tri_subs, mask_subs): j = key chunk, d0 = first valid 128-q
    sub-block, tri_subs = subs using the generated triangular mask,
    mask_subs = (d, block_id) pairs using DMA'd mask blocks.
    """
    m = np.asarray(mask)
    assert m.shape == (N, N)
    tril = np.tril(np.ones((N, N), m.dtype))
    if np.array_equal(m, tril):
        sched = []
        for qb in range(NQB):
            ent = []
            for j in range(4 * qb + 4):
                t = j - 4 * qb
                if t < 0:
                    ent.append((j, 0, [], []))
                else:
                    ent.append((j, t, [t], []))
            sched.append(ent)
        return "causal", sched, None
    if np.all(m == 1):
        sched = [[(j, 0, [], []) for j in range(NKC)] for _ in range(NQB)]
        return "full", sched, None
    # General: classify 128x128 blocks of mask.T (rows=key, cols=query).
    mt = m.T
    blocks = {}
    packed = []

    def block_id(blk):
        key = blk.tobytes()
        if key not in blocks:
            blocks[key] = len(packed)
            packed.append(blk.astype(BF16))
        return blocks[key]

    sched = []
    for qb in range(NQB):
        ent = []
        for j in range(NKC):
            subs = []
            for d in range(4):
                blk = mt[j * 128:(j + 1) * 128,
                         qb * 512 + d * 128:qb * 512 + (d + 1) * 128]
                if np.all(blk == 0):
                    subs.append(("skip", None))
                elif np.all(blk == 1):
                    subs.append(("full", None))
                else:
                    subs.append(("mask", block_id(blk)))
            if all(s[0] == "skip" for s in subs):
                continue
            d0 = min(d for d, s in enumerate(subs) if s[0] != "skip")
            mask_subs = [(d, s[1]) for d, s in enumerate(subs) if s[0] == "mask"]
            for d in range(d0, 4):
                if subs[d][0] == "skip":
                    mask_subs.append((d, block_id(np.zeros((128, 128)))))
            ent.append((j, d0, [], sorted(mask_subs)))
        sched.append(ent)
    mask_pack = np.concatenate(packed, axis=1) if packed else None
    return "general", sched, mask_pack


def _build_program(sched, n_mask_blocks, use_bias):
    import concourse.mybir as mybir
    import concourse.tile as tile
    from concourse import bacc
    from concourse.masks import make_identity, make_upper_triangular

    bf = mybir.dt.bfloat16
    f32 = mybir.dt.float32
    Exp = mybir.ActivationFunctionType.Exp
    nc = bacc.Bacc(None, target_bir_lowering=False, num_devices=NCORES)

    # This core's 128-dim slice of x.T for q/k/v, stacked: rows 0:128 =
    # query.T slice, 128:256 = key.T slice, 256:384 = value.T slice.
    xTs = nc.dram_tensor("xTs", [3 * PD, BN], bf, kind="ExternalInput")
    wT = {n: nc.dram_tensor(n, [128, 8 * PD], bf, kind="ExternalInput")
          for n in ("wq", "wk", "wv")}
    woT = nc.dram_tensor("woT", [128, 8 * 128], bf, kind="ExternalInput")
    if use_bias:
        bqkv = nc.dram_tensor("bqkv", [PD, 3], f32, kind="ExternalInput")
    if n_mask_blocks:
        maskblk = nc.dram_tensor("maskblk", [128, n_mask_blocks * 128], bf,
                                 kind="ExternalInput")
    outp = nc.dram_tensor("outp", [128, BN], bf, kind="ExternalOutput")

    rg = [list(range(NCORES))]
    Bypass = mybir.AluOpType.bypass

    with tile.TileContext(nc) as tc, ExitStack() as st_:
        dram = st_.enter_context(tc.tile_pool(name="dram", bufs=1,
                                              space="DRAM"))
        ag_in_qk = [dram.tile([2 * PD, N], bf, name=f"ag_in_qk{i}")
                    for i in range(B)]
        ag_in_v = [dram.tile([PD, N], bf, name=f"ag_in_v{i}")
                   for i in range(B)]
        ag_out_qk = [dram.tile([NCORES * 2 * PD, N], bf,
                               addr_space="Shared", name=f"ag_out_qk{i}")
                     for i in range(B)]
        ag_out_v = [dram.tile([NCORES * PD, N], bf, addr_space="Shared",
                              name=f"ag_out_v{i}")
                    for i in range(B)]
        agc_in = [dram.tile([128, N], bf, name=f"agc_in{i}")
                  for i in range(B)]
        agc_out = [dram.tile([NCORES * 128, N], bf, addr_space="Shared",
                             name=f"agc_out{i}")
                   for i in range(B)]

        # Pipelined AllGathers of the x.T dim-slices, per batch-half and
        # split q+k vs v: batch 0's q+k gather is a quarter of the full x,
        # so its projections and attention start ~2x earlier; every later
        # gather hides under batch 0's compute. Rank r's q|k block lands at
        # rows [r*256:(r+1)*256] of ag_out_qk[b]; its v slice at rows
        # [r*128:(r+1)*128] of ag_out_v[b].
        for b_ in range(B):
            nc.sync.dma_start(out=ag_in_qk[b_][:, :],
                              in_=xTs[0:2 * PD, b_ * N:(b_ + 1) * N])
            nc.sync.dma_start(out=ag_in_v[b_][:, :],
                              in_=xTs[2 * PD:3 * PD, b_ * N:(b_ + 1) * N])
        for b_ in range(B):
            nc.gpsimd.collective_compute(
                "AllGather", Bypass, replica_groups=rg,
                ins=[ag_in_qk[b_].opt()], outs=[ag_out_qk[b_].opt()])
            nc.gpsimd.collective_compute(
                "AllGather", Bypass, replica_groups=rg,
                ins=[ag_in_v[b_].opt()], outs=[ag_out_v[b_].opt()])

        singles = st_.enter_context(tc.tile_pool(name="singles", bufs=1))

        ident = singles.tile([128, 128], bf)
        make_identity(nc, ident[:, :])
        tri = singles.tile([128, 128], bf)
        make_upper_triangular(nc, tri[:, :], val=1.0, diag=True)
        ones = singles.tile([128, 128], bf)
        nc.vector.memset(ones[:, :], 1.0)

        w_sb = {}
        for n in ("wq", "wk", "wv"):
            w_sb[n] = singles.tile([128, 8 * PD], bf, name=f"w_{n}")
            nc.sync.dma_start(out=w_sb[n][:, :], in_=wT[n][:, :])
        wo_sb = singles.tile([128, 8 * 128], bf)
        nc.sync.dma_start(out=wo_sb[:, :], in_=woT[:, :])
        if use_bias:
            b_sb = singles.tile([128, 3], f32)
            nc.sync.dma_start(out=b_sb[:, :], in_=bqkv[:, :])
        if n_mask_blocks:
            mask_sb = singles.tile([128, n_mask_blocks * 128], bf)
            nc.sync.dma_start(out=mask_sb[:, :], in_=maskblk[:, :])

        qTs = [singles.tile([128, N], bf, name=f"qT{i}") for i in range(B)]
        kTs = [singles.tile([128, N], bf, name=f"kT{i}") for i in range(B)]
        vTs = [singles.tile([128, N], bf, name=f"vT{i}") for i in range(B)]
        # v_aug holds 130 columns per 128-key chunk: [vA(64)|1|vB(64)|1] —
        # the ones columns ride along as stationary col 64 of each head's
        # PV matmul, producing the softmax denominator in PSUM row 64 at
        # zero extra PE cost (matmul cost is moving-width cycles only).
        v_augs = [singles.tile([128, NKC * 130], bf, name=f"vaug{i}")
                  for i in range(B)]
        # per-head ctx.T halves at partition base 0 (head B's PSUM rows
        # 0:64 copy out partition-aligned; the DMA into the gather buffer
        # restacks them into rows 0:64 / 64:128).
        ctxhs = [[singles.tile([64, N], bf, name=f"ctx{i}h{h}")
                  for h in range(2)] for i in range(B)]
        outTs = [singles.tile([128, N], bf, name=f"outT{i}") for i in range(B)]

        xp = st_.enter_context(tc.tile_pool(name="xp", bufs=16))
        cg = st_.enter_context(tc.tile_pool(name="cg", bufs=8))
        ptile = st_.enter_context(tc.tile_pool(name="ptile", bufs=4))
        rp = st_.enter_context(tc.tile_pool(name="rp", bufs=2))

        # ---- projections, per batch-half ----
        # Batch 0's run up front (right after its quarter-size gathers);
        # batch 1's loads and accumulation groups are deferred into batch
        # 0's attention chunk loop, hiding its gathers and matmuls there.
        xts_all = {}

        def emit_xloads(b, ti, qeng):
            n = ("xq", "xk", "xv")[ti]
            tiles = []
            for c in range(8):
                xt = xp.tile([128, N], bf, tag="x", name=f"xt{b}{n}{c}")
                if ti < 2:
                    src = ag_out_qk[b][c * 2 * PD + ti * PD:
                                       c * 2 * PD + (ti + 1) * PD, :]
                else:
                    src = ag_out_v[b][c * PD:(c + 1) * PD, :]
                qeng.dma_start(out=xt[:, :], in_=src)
                tiles.append(xt)
            xts_all[(b, ti)] = tiles

        def emit_proj_acc(b, ti, blk, pool):
            n = ("xq", "xk", "xv")[ti]
            w = w_sb["w" + n[1]]
            xts = xts_all[(b, ti)]
            acc = pool.tile([128, 512], f32, tag="acc", name="acc")
            for c in range(8):
                nc.tensor.matmul(
                    acc[:, :],
                    w[:, c * PD:(c + 1) * PD],
                    xts[c][:, blk * 512:(blk + 1) * 512],
                    start=(c == 0), stop=(c == 7))
            dst = (qTs, kTs, vTs)[ti][b][:, blk * 512:(blk + 1) * 512]
            if use_bias:
                nc.vector.tensor_scalar_add(dst, acc[:, :],
                                            b_sb[:, ti:ti + 1])
            else:
                nc.vector.tensor_copy(dst, acc[:, :])

        with tc.tile_pool(name="pp8", bufs=8, space="PSUM") as pp8:
            # k0/v0 ride the scalar queue (free until the first exp), q0
            # the sync queue; batch 1's loads go to sync/gpsimd later so
            # they never block batch 0's exp stream on the scalar queue.
            emit_xloads(0, 0, nc.sync)
            emit_xloads(0, 1, nc.scalar)
            emit_xloads(0, 2, nc.scalar)
            for ti in range(3):
                for blk in range(4):
                    emit_proj_acc(0, ti, blk, pp8)

        pending_proj = [("load", 0, nc.sync), ("load", 1, nc.sync),
                        ("load", 2, nc.gpsimd)]
        pending_proj += [("acc", ti, blk)
                         for ti in range(3) for blk in range(4)]

        ps = st_.enter_context(tc.tile_pool(name="ps", bufs=2, space="PSUM"))
        po = st_.enter_context(tc.tile_pool(name="po", bufs=2, space="PSUM"))
        pp = st_.enter_context(tc.tile_pool(name="pp", bufs=2, space="PSUM"))

        # deferred output-projection work: (batch, 512-token block) pairs
        # whose ctx AllGather has been issued.
        pending_op = []
        ctxg = {}
        # deferred softmax-normalization tails: the PE-side replication
        # matmul is emitted one chunk into the next q-block so the PE queue
        # never stalls waiting on the l-row copy.
        pending_norm = []

        def emit_norm(b, qc0, ovh, lbs):
            for hh in range(2):
                lrep = pp.tile([64, 512], f32, tag="acc", name="lrep")
                nc.tensor.matmul(
                    lrep[:, :], ones[64:65, 0:64], lbs[hh][64:65, :],
                    start=True, stop=True)
                rc = rp.tile([64, 512], f32, tag="rc")
                nc.vector.reciprocal_approx_fast(
                    out=rc[:, :], in_=lrep[:, :])
                nc.vector.tensor_mul(
                    ctxhs[b][hh][:, qc0:qc0 + 512],
                    ovh[hh][0:64, 0:512], rc[:, :])

        def emit_ctx_gather(b):
            # ship this batch's ctx.T to DRAM, AllGather, and pull the full
            # [1024, N] gathered ctx.T back into SBUF chunk tiles.
            nc.sync.dma_start(out=agc_in[b][0:64, :], in_=ctxhs[b][0][:, :])
            nc.sync.dma_start(out=agc_in[b][64:128, :], in_=ctxhs[b][1][:, :])
            nc.gpsimd.collective_compute(
                "AllGather", Bypass, replica_groups=rg,
                ins=[agc_in[b].opt()], outs=[agc_out[b].opt()])
            tiles = []
            for c in range(8):
                t = cg.tile([128, N], bf, tag="cg", name=f"cg{b}{c}")
                qeng = (nc.gpsimd, nc.scalar, nc.sync)[c % 3]
                qeng.dma_start(
                    out=t[:, :], in_=agc_out[b][c * 128:(c + 1) * 128, :])
                tiles.append(t)
            ctxg[b] = tiles
            pending_op.extend((b, tb) for tb in range(0, N // 512, 2))

        def emit_oproj(b, tb0):
            # out columns [rank*128 : rank*128+128] for 1024 tokens of
            # batch b: two 512-token PSUM accumulators advanced chunk-major
            # over the 8 gathered ctx chunks (one weight load serves both
            # accumulators, and the first matmuls only need chunk 0).
            ops = [pp.tile([128, 512], f32, tag="acc", name=f"op{i}")
                   for i in range(2)]
            for c in range(8):
                for i in range(2):
                    nc.tensor.matmul(
                        ops[i][:, :],
                        wo_sb[:, c * 128:(c + 1) * 128],
                        ctxg[b][c][:, (tb0 + i) * 512:(tb0 + i + 1) * 512],
                        start=(c == 0), stop=(c == 7))
            for i in range(2):
                dst = outTs[b][:, (tb0 + i) * 512:(tb0 + i + 1) * 512]
                if i == 0:
                    nc.vector.tensor_copy(dst, ops[i][:, :])
                else:
                    nc.scalar.copy(dst, ops[i][:, :])
            if tb0 + 2 >= N // 512:
                # last block of this batch: ship its outT slice now so the
                # store overlaps whatever compute remains.
                nc.sync.dma_start(out=outp[:, b * N:(b + 1) * N],
                                  in_=outTs[b][:, :])

        for b in range(B):
            h0 = b * N
            qT, kT, vT = qTs[b], kTs[b], vTs[b]
            v_aug = v_augs[b]
            # ---- v transpose into v_aug (ones columns pre-set) ----
            nc.vector.memset(v_aug[:, :], 1.0)
            for t in range(N // 128):
                vtp = ps.tile([128, 128], bf, tag="st", name="vtp")
                nc.tensor.transpose(
                    vtp[:, :], vT[:, t * 128:(t + 1) * 128], ident[:, :])
                nc.vector.tensor_copy(
                    v_aug[:, t * 130:t * 130 + 64], vtp[:, 0:64])
                nc.vector.tensor_copy(
                    v_aug[:, t * 130 + 65:t * 130 + 129], vtp[:, 64:128])

            # ---- attention; prior batch's output projection interleaves ----
            for qb in range(NQB):
                qc0 = qb * 512
                ent = sched[qb]
                if len(ent) < 2:
                    # too few chunks to hide the deferred tail behind —
                    # drain before this q-block's PSUM tiles recycle it.
                    while pending_norm:
                        emit_norm(*pending_norm.pop(0))
                ovh = [po.tile([65, 512], f32, tag="ov", name=f"ov{h}")
                       for h in range(2)]

                def emit_st(e):
                    j, d0, _, _ = e
                    kc0 = j * 128
                    c0 = d0 * 128
                    stt = ps.tile([128, 1024], f32, tag="st", name="stt")
                    nc.tensor.matmul(
                        stt[:, c0:512],
                        kT[0:64, kc0:kc0 + 128],
                        qT[0:64, qc0 + c0:qc0 + 512],
                        start=True, stop=True)
                    nc.tensor.matmul(
                        stt[:, 512 + c0:1024],
                        kT[64:128, kc0:kc0 + 128],
                        qT[64:128, qc0 + c0:qc0 + 512],
                        start=True, stop=True)
                    return stt

                def emit_rest(e, stt, first, last):
                    j, d0, tri_subs, mask_subs = e
                    kc0 = j * 128
                    c0 = d0 * 128
                    pte = ptile.tile([128, 1024], bf, tag="pt", name="pte")
                    nc.scalar.activation(
                        pte[:, :].rearrange("p (h c) -> p h c", h=2)
                           [:, :, c0:512],
                        stt[:, :].rearrange("p (h c) -> p h c", h=2)
                           [:, :, c0:512],
                        Exp, scale=SCALE)
                    for d in tri_subs:
                        for hh in range(2):
                            pv = pte[:, hh * 512 + d * 128:
                                     hh * 512 + (d + 1) * 128]
                            nc.vector.tensor_mul(pv, pv, tri[:, :])
                    for (d, blkid) in mask_subs:
                        for hh in range(2):
                            pv = pte[:, hh * 512 + d * 128:
                                     hh * 512 + (d + 1) * 128]
                            nc.vector.tensor_mul(
                                pv, pv,
                                mask_sb[:, blkid * 128:(blkid + 1) * 128])
                    vb0 = j * 130
                    for hh in range(2):
                        pr = pte[:, hh * 512 + c0:(hh + 1) * 512]
                        nc.tensor.matmul(
                            ovh[hh][0:65, c0:512],
                            v_aug[:, vb0 + hh * 65:vb0 + (hh + 1) * 65], pr,
                            start=first, stop=last,
                            skip_group_check=True)

                # software pipeline: S.T matmuls run one chunk ahead of the
                # exp/mask/PV stage so PE never waits on ScalarE.
                pend = None
                for idx, e in enumerate(ent):
                    stt = emit_st(e)
                    if idx == 1 and pending_norm:
                        emit_norm(*pending_norm.pop(0))
                    if pend is not None:
                        emit_rest(pend[0], pend[1], pend[2], False)
                    # pop deferred cross-batch work only once the gathers/
                    # loads it depends on have had time to land, so the
                    # in-order PE queue never stalls: batch 1's projections
                    # into batch 0's attention, batch 0's output projection
                    # into batch 1's.
                    if b == 0 and qb >= 1 and pending_proj:
                        it = pending_proj.pop(0)
                        if it[0] == "load":
                            emit_xloads(1, it[1], it[2])
                        else:
                            emit_proj_acc(1, it[1], it[2], pp)
                    if qb >= 2 and idx % 2 == 1 and pending_op:
                        emit_oproj(*pending_op.pop(0))
                    pend = (e, stt, idx == 0)
                emit_rest(pend[0], pend[1], pend[2], True)
                # normalize: row 64 of each head's PSUM holds the softmax
                # denominator. Copy it out on DVE now; the replicate/
                # reciprocal/multiply tail is deferred into the next
                # q-block's chunk loop.
                lbs = []
                for hh in range(2):
                    lb = rp.tile([65, 512], bf, tag="lb", name=f"lb{hh}")
                    nc.vector.tensor_copy(lb[64:65, :], ovh[hh][64:65, :])
                    lbs.append(lb)
                pending_norm.append((b, qc0, ovh, lbs))
            while pending_norm:
                emit_norm(*pending_norm.pop(0))
            while pending_proj:
                it = pending_proj.pop(0)
                if it[0] == "load":
                    emit_xloads(1, it[1], it[2])
                else:
                    emit_proj_acc(1, it[1], it[2], pp)
            # this batch's ctx is complete: gather it across cores; its
            # output projection interleaves into the next batch's chunk
            # loop (or drains below for the last batch).
            emit_ctx_gather(b)

        while pending_op:
            emit_oproj(*pending_op.pop(0))
    nc.compile()
    return nc


def _prep_in_maps(query, key, value, Wq, Wk, Wv, Wo, bq, bk, bv,
                  use_bias, mask_pack):
    def prep_xT(x):
        return np.ascontiguousarray(
            np.asarray(x, np.float32).reshape(BN, D).T).astype(BF16)

    def prep_w(W, r0, r1):
        # SBUF layout [128, 8*PD]: [p, c*PD+m] = W.T[c*128+p, m]
        wt = np.asarray(W, np.float32)[r0:r1, :].T  # [D, PD]
        pd = r1 - r0
        return np.ascontiguousarray(
            wt.reshape(8, 128, pd).transpose(1, 0, 2).reshape(128, 8 * pd)
        ).astype(BF16)

    xqT, xkT, xvT = prep_xT(query), prep_xT(key), prep_xT(value)
    in_maps = []
    for c in range(NCORES):
        r0, r1 = c * PD, (c + 1) * PD
        m = {
            "xTs": np.ascontiguousarray(
                np.concatenate([xqT[r0:r1], xkT[r0:r1], xvT[r0:r1]], axis=0)),
            "wq": prep_w(Wq, r0, r1),
            "wk": prep_w(Wk, r0, r1),
            "wv": prep_w(Wv, r0, r1),
            "woT": prep_w(Wo, c * 128, (c + 1) * 128),
        }
        if use_bias:
            m["bqkv"] = np.ascontiguousarray(np.stack(
                [np.asarray(bq)[r0:r1], np.asarray(bk)[r0:r1],
                 np.asarray(bv)[r0:r1]], axis=1)).astype(np.float32)
        if mask_pack is not None:
            m["maskblk"] = np.ascontiguousarray(mask_pack)
        in_maps.append(m)
    return in_maps


def kernel(query, key, value, mask, Wq, bq, Wk, bk, Wv, bv, Wo, bo):
    from concourse.bass_utils import run_bass_kernel_spmd

    mode, sched, mask_pack = _mask_schedule(mask)
    n_mask_blocks = 0 if mask_pack is None else mask_pack.shape[1] // 128
    use_bias = bool(np.any(bq) or np.any(bk) or np.any(bv))
    nc = _build_program(sched, n_mask_blocks, use_bias)
    in_maps = _prep_in_maps(query, key, value, Wq, Wk, Wv, Wo, bq, bk, bv,
                            use_bias, mask_pack)
    res = run_bass_kernel_spmd(nc, in_maps, core_ids=list(range(NCORES)))
    outT = np.concatenate([r["outp"] for r in res.results], axis=0)
    out = outT.astype(np.float32).T + np.asarray(bo, np.float32)
    return out.reshape(B, N, D)
